# revision 1
# baseline (speedup 1.0000x reference)
"""GNN attention (GAT-style single-target-node) kernel for 8 Trainium2 cores.

Problem:  x [32, 50000, 64], a [128, 1], node_index scalar, adj_mask [50000]
  tgt_score = x[:, idx] @ a[:64]                             # [B]
  e = leaky_relu(tgt_score[:, None] + x @ a[64:], 0.01)      # [B, N]
  attention = softmax(where(adj>0, e, -9e15), axis=1) * adj  # [B, N]

Sharding: data-parallel over batch (32 = 8 cores x 4 batches/core), the 4
batches paired into 2 batch-pairs.  Each core computes complete softmax
rows, so no cross-core reductions.

All dot products run on the otherwise-idle PE: the host lays x out as
xh[pair, bi*64+d, col] (fp16, both batches of a pair stacked on the
contraction axis), each 128-node chunk of xh is the *stationary* operand
[K=128, M=128 nodes], and a tiny constant a-matrix [128, 2] streams as the
moving operand, so LDWEIGHTS itself is the data pass (128 values/cycle) and
out = [128 nodes, 2 batches] lands dense in PSUM.  Host column permutation
col = c*128 + p <-> node p*391 + c makes the final attention write
contiguous per partition.  DVE only does the short softmax tail, reading
scores straight out of PSUM; fp16 x halves HBM traffic (the roofline).
"""

import numpy as np
from contextlib import ExitStack

import jax
from jax.sharding import Mesh, PartitionSpec
from jax.experimental.shard_map import shard_map

import concourse.bass as bass
import concourse.bass_isa as bass_isa
import concourse.bacc as bacc
import concourse.tile as tile
from concourse.tile import add_dep_helper
from concourse import mybir
from concourse.bass2jax import _bass_exec_p, install_neuronx_cc_hook

B, N, D = 32, 50000, 64
NCORES = 8
BPC = B // NCORES            # batches per core
PAIRS = BPC // 2             # batch-pairs per core
CHUNKS = 391                 # 128-node chunks per batch: 128*391 = 50048
PADN = 128 * CHUNKS          # padded node count (48 pad nodes)
W = 2 * CHUNKS               # score-grid cols per pair: (chunk, batch) pairs
BANK = 512                   # f32 cols per PSUM bank
CPB = BANK // 2              # chunks per PSUM bank (256)
TILE_F = 16384               # xh cols per DMA tile (4 MB fp16, 128 chunks)
CPT = TILE_F // 128          # chunks per full tile (128)
NMAIN = 127 * CHUNKS         # nodes covered by the [127, 391] output DMA
NTAIL = N - NMAIN            # 343 nodes in partition 127
XB = 4                       # x-tile pool depth
RING_SPLIT = False           # alternate x-tile DMAs across sync/scalar HWDGE
TILES_F = (16384, 16384, 16384, 896)   # per-pair DMA tile widths (cols)
NEG = -9.0e15

F32 = mybir.dt.float32
F16 = mybir.dt.float16
AX = mybir.AxisListType
OP = mybir.AluOpType
ACT = mybir.ActivationFunctionType

TRACE = False
LAST_RUN = None

_CACHE = {}


def _build(reps=1, hw_loop=1, mode="full"):
    """reps: python-unrolled kernel bodies; hw_loop > 1 additionally wraps
    them in a hardware For_i loop (total bodies = reps * hw_loop) so timing
    NEFFs can amortize the ~100 ms (+/- tens of ms) axon dispatch jitter over
    hundreds of bodies without exploding the instruction count.

    mode: diagnostic bodies for attributing HW time -- "full" (the real
    kernel), "dma" (x DMA stream only), "pe" (matmuls+softmax from a static
    SBUF tile, no x DMAs), "both" (DMA stream + static-tile matmuls: no
    data dependency between the two streams)."""
    nc = bacc.Bacc(trn_type="TRN2", enable_partition_id=False,
                   num_devices=NCORES)
    xs = nc.dram_tensor("xs", [PAIRS, 128, PADN], F16,
                        kind="ExternalInput").ap()
    amov_d = nc.dram_tensor("amov", [128, 2], F16, kind="ExternalInput").ap()
    tgtg_d = nc.dram_tensor("tgtg", [PAIRS, 128, W], F32,
                            kind="ExternalInput").ap()
    mb_d = nc.dram_tensor("mbgrid", [128, W], F32, kind="ExternalInput").ap()
    attn = nc.dram_tensor("attn", [BPC, N], F32, kind="ExternalOutput").ap()

    tiles_f = list(TILES_F)
    assert sum(tiles_f) == PADN

    with tile.TileContext(nc) as tc, ExitStack() as ctx:
        singles = ctx.enter_context(tc.tile_pool(name="singles", bufs=1))
        xpool = ctx.enter_context(tc.tile_pool(name="xpool", bufs=XB))
        gpool = ctx.enter_context(tc.tile_pool(name="gpool", bufs=2))
        epool = ctx.enter_context(tc.tile_pool(name="epool", bufs=2))
        stat = ctx.enter_context(tc.tile_pool(name="stat", bufs=8))
        psco = ctx.enter_context(tc.tile_pool(name="psco", bufs=2,
                                              space="PSUM"))

        amov_sb = singles.tile([128, 2], F16)
        with tc.high_priority():
            nc.sync.dma_start(out=amov_sb, in_=amov_d)
        mb_sb = singles.tile([128, W], F32)
        nc.scalar.dma_start(out=mb_sb, in_=mb_d)
        tgtg_sb = singles.tile([128, PAIRS * W], F32)
        for j in range(PAIRS):
            nc.scalar.dma_start(out=tgtg_sb[:, j * W:(j + 1) * W],
                              in_=tgtg_d[j])
        tgtg_v = [tgtg_sb[:, j * W:(j + 1) * W] for j in range(PAIRS)]
        state = {"first_tile": True}

        def body():
            for _ in range(reps):
                _one_rep()

        xstat = None
        if mode in ("pe", "both"):
            xstat = singles.tile([128, TILE_F], F16)
            nc.vector.memset(xstat, 0.0)

        def _one_rep():
            for j in range(PAIRS):
                # --- load xh tiles and run one matmul per 128-node chunk ---
                xts = []
                f0 = 0
                for t, tf in enumerate(tiles_f):
                    f1 = f0 + tf
                    if mode == "pe":
                        f0 = f1
                        continue
                    xt = xpool.tile([128, tf], F16)
                    if state["first_tile"]:
                        # quarter the very first DMA so PE starts ~4x earlier
                        state["first_tile"] = False
                        q = (f1 - f0) // 4
                        with tc.high_priority():
                            for i in range(4):
                                nc.sync.dma_start(
                                    out=xt[:, i * q:(i + 1) * q],
                                    in_=xs[j, :, f0 + i * q:f0 + (i + 1) * q])
                    else:
                        eng = nc.scalar if (RING_SPLIT and t % 2) else nc.sync
                        eng.dma_start(out=xt, in_=xs[j, :, f0:f1])
                    xts.append((f0, xt))
                    f0 = f1
                if mode == "dma":
                    continue

                ps0 = psco.tile([128, BANK], F32, tag="sc0")
                ps1 = psco.tile([128, BANK], F32, tag="sc1")
                ps = [ps0, ps1]
                for c in range(CHUNKS):
                    col = c * 128
                    if mode in ("pe", "both"):
                        cm = col % (TILE_F - 128)
                        lhsT = xstat[:, cm:cm + 128]
                    else:
                        tf0, xt = next((f0, x) for f0, x in reversed(xts)
                                       if f0 <= col)
                        lhsT = xt[:, col - tf0:col - tf0 + 128]
                    bank, cb = divmod(c, CPB)
                    nc.tensor.matmul(ps[bank][:, 2 * cb:2 * cb + 2],
                                     lhsT, amov_sb, start=True, stop=True)

                # --- softmax tail, straight out of PSUM ---
                # z = leaky_relu(scores + tgt, 0.01) + mask_bias.  Scores are
                # O(10): exp cannot overflow fp32, so no max-subtraction.
                z = epool.tile([128, W], F32)
                nc.vector.tensor_add(z[:, :BANK], ps[0], tgtg_v[j][:, :BANK])
                nc.vector.tensor_add(z[:, BANK:], ps[1][:, :W - BANK],
                                     tgtg_v[j][:, BANK:])
                nc.vector.scalar_tensor_tensor(z, z, 0.01, z,
                                               op0=OP.mult, op1=OP.max)
                nc.vector.tensor_add(z, z, mb_sb)

                pbc = epool.tile([128, 2, CHUNKS], F32, tag="pbc")
                for bi in range(2):
                    b = 2 * j + bi
                    # exp with per-partition row sums; global sum + broadcast
                    # in ONE idle-GPSIMD op (daisy-chain all-reduce) instead
                    # of the PE-transpose / ones-matmul round trip.
                    srow = stat.tile([128, 1], F32)
                    nc.scalar.activation(pbc[:, bi, :], z[:, bi::2], ACT.Exp,
                                         bias=0.0, scale=1.0, accum_out=srow)
                    gsum = stat.tile([128, 1], F32, tag="gsum")
                    nc.gpsimd.partition_all_reduce(gsum, srow, 128,
                                                   bass_isa.ReduceOp.add)
                    rec = stat.tile([128, 1], F32, tag="rec")
                    nc.vector.reciprocal(rec, gsum)
                    nc.vector.tensor_scalar_mul(pbc[:, bi, :], pbc[:, bi, :],
                                                rec)
                    nc.scalar.dma_start(
                        out=attn[b, 0:NMAIN].rearrange("(p c) -> p c",
                                                       c=CHUNKS),
                        in_=pbc[0:127, bi, :])
                    nc.scalar.dma_start(
                        out=attn[b, NMAIN:N].rearrange("(o c) -> o c", o=1),
                        in_=pbc[127:128, bi, 0:NTAIL])

        if hw_loop > 1:
            with tc.For_i(0, hw_loop):
                body()
        else:
            body()
    nc.compile()
    return nc


def _host_prep(x, a, node_index, adj_mask):
    x = np.asarray(x, dtype=np.float32)
    a = np.asarray(a, dtype=np.float32).reshape(2 * D)
    adj = np.asarray(adj_mask).astype(np.int64)
    idx = int(node_index)
    a_tgt, a_src = a[:D], a[D:]

    tgt = (x[:, idx, :] @ a_tgt).astype(np.float32)          # [B]

    # xh[pair, bi*64+d, c*128+p] = x[2*pair+bi, p*391+c, d]  (fp16, 0-padded)
    perm = (np.arange(CHUNKS)[:, None] + np.arange(128)[None, :] * CHUNKS)
    perm = perm.ravel()                                      # col -> node id
    xt16 = np.ascontiguousarray(x.transpose(0, 2, 1), dtype=np.float16)
    xtp = np.concatenate(
        [xt16, np.zeros((B, D, PADN - N), np.float16)], axis=2)
    xh = np.ascontiguousarray(xtp[:, :, perm]).reshape(B // 2, 128, PADN)

    amov = np.zeros((128, 2), np.float16)
    amov[0:D, 0] = a_src
    amov[D:2 * D, 1] = a_src

    # mask-bias in (p, 2c+bi) layout; pad nodes (>= N) get NEG
    mb1 = np.full(PADN, NEG, np.float32)
    valid = perm < N
    mb1[valid] = np.where(adj[perm[valid]] > 0, 0.0, NEG)
    mbg = np.repeat(mb1.reshape(CHUNKS, 128).T[:, :, None], 2,
                    axis=2).reshape(128, W)

    # tgt grid [B//2 pairs, 128, W]: value tgt[2*pair+bi] in cols 2c+bi
    tgtg = np.ascontiguousarray(np.broadcast_to(
        tgt.reshape(B // 2, 1, 1, 2), (B // 2, 128, CHUNKS, 2))
    ).reshape(B // 2, 128, W)

    return xh, amov, tgtg, mbg


def _in_maps(xh, amov, tgtg, mbg):
    maps = []
    for c in range(NCORES):
        maps.append({
            "xs": xh[c * PAIRS:(c + 1) * PAIRS],
            "amov": amov,
            "tgtg": tgtg[c * PAIRS:(c + 1) * PAIRS],
            "mbgrid": mbg,
        })
    return maps


def _runner():
    """Build the Bass program once and wrap its NEFF custom call in a jitted
    shard_map over the 8 cores."""
    if "runner" in _CACHE:
        return _CACHE["runner"]
    install_neuronx_cc_hook()
    nc = _CACHE.setdefault("nc", _build())
    in_names, out_names, out_avals, zero_shapes = [], [], [], []
    for alloc in nc.m.functions[0].allocations:
        if not isinstance(alloc, mybir.MemoryLocationSet):
            continue
        name = alloc.memorylocations[0].name
        if alloc.kind == "ExternalInput":
            in_names.append(name)
        elif alloc.kind == "ExternalOutput":
            out_names.append(name)
            shape = tuple(alloc.tensor_shape)
            dtype = mybir.dt.np(alloc.dtype)
            out_avals.append(jax.core.ShapedArray(shape, dtype))
            zero_shapes.append((shape, dtype))

    def _body(*args):
        return tuple(_bass_exec_p.bind(
            *args,
            out_avals=tuple(out_avals),
            in_names=tuple(in_names + out_names),
            out_names=tuple(out_names),
            lowering_input_output_aliases=(),
            sim_require_finite=True,
            sim_require_nnan=True,
            nc=nc,
        ))

    mesh = Mesh(np.asarray(jax.devices()[:NCORES]), ("core",))
    nin = len(in_names) + len(out_names)
    sharded = jax.jit(shard_map(
        _body, mesh=mesh,
        in_specs=(PartitionSpec("core"),) * nin,
        out_specs=(PartitionSpec("core"),) * len(out_names),
        check_rep=False))
    _CACHE["runner"] = (sharded, in_names, out_names, zero_shapes)
    return _CACHE["runner"]


def kernel(x, a, node_index, adj_mask):
    global LAST_RUN
    prep = _host_prep(x, a, node_index, adj_mask)
    maps = _in_maps(*prep)
    sharded, in_names, out_names, zero_shapes = _runner()
    # concat of the 8 per-core xs/tgtg shards is exactly the full arrays
    full = {"xs": prep[0], "tgtg": prep[2]}
    ins = [full[nm] if nm in full else
           np.concatenate([m[nm] for m in maps], axis=0) for nm in in_names]
    zeros = [np.zeros((NCORES * s[0], *s[1:]), d) for s, d in zero_shapes]
    outs = sharded(*ins, *zeros)
    LAST_RUN = outs
    attn = np.asarray(outs[out_names.index("attn")])  # [NCORES*BPC, N]
    return attn.reshape(B, N)



# revision 3
# speedup vs baseline: 2.4830x; 2.4830x over previous
"""GNN attention (GAT-style single-target-node) kernel for 8 Trainium2 cores.

Problem:  x [32, 50000, 64], a [128, 1], node_index scalar, adj_mask [50000]
  tgt_score = x[:, idx] @ a[:64]                             # [B]
  e = leaky_relu(tgt_score[:, None] + x @ a[64:], 0.01)      # [B, N]
  attention = softmax(where(adj>0, e, -9e15), axis=1) * adj  # [B, N]

Key observation: rows with adj_mask == 0 contribute exp(-9e15) = 0 to the
softmax denominator and their output is *exactly* 0 (softmax * adj).  So the
device only ever needs x at the ~25k unmasked nodes: the host compacts
x[:, keep, :] (fp16) before upload and scatters the compact attention back
into a zero [B, N] canvas afterwards.  This halves HBM traffic vs the dense
kernel and removes the mask-bias grid entirely.

Sharding: data-parallel over batch (32 = 8 cores x 4 batches/core), the 4
batches paired into 2 batch-pairs.  Each core computes complete softmax rows,
so no cross-core reductions.

Per pair the host lays compact x out as xh[pair, bi*64+d, col] (fp16, both
batches of a pair stacked on the contraction axis).  Each 128-node chunk is
the *stationary* matmul operand [K=128, M=128 nodes] and the tiny constant
a-matrix [128, 2] streams as the moving operand, so LDWEIGHTS itself is the
data pass (128 values/cycle) and out = [128 nodes, 2 batches] lands
interleaved in ONE PSUM bank (2*CH <= 512).  Column permutation
col = c*128 + p <-> compact slot p*CH + c makes the final attention write
contiguous per partition.  Pad slots (>= M) carry a host-built vector w with
w @ a_src = -30000 so they vanish under exp() with no mask read or multiply.
The per-batch tgt_score is added as a [128,1] per-partition DVE scalar
operand (2 KB) instead of a broadcast [128, W] grid (800 KB)."""

import numpy as np
from contextlib import ExitStack

import jax
from jax.sharding import Mesh, PartitionSpec
from jax.experimental.shard_map import shard_map

import concourse.bass as bass
import concourse.bass_isa as bass_isa
import concourse.bacc as bacc
import concourse.tile as tile
from concourse import mybir
from concourse.bass2jax import _bass_exec_p, install_neuronx_cc_hook

B, N, D = 32, 50000, 64
NCORES = 8
BPC = B // NCORES            # batches per core
PAIRS = BPC // 2             # batch-pairs per core
CH = 195                     # 128-node chunks per batch: 128*195 = 24960 >= M
NTILES = 4                   # xh DMA tiles per pair
XB = 6                       # x-tile pool depth
RING_SPLIT = False           # alternate x-tile DMAs across sync/scalar HWDGE
NEGW = -30000.0              # pad-slot score (vanishes under exp after lrelu)

F32 = mybir.dt.float32
F16 = mybir.dt.float16
AX = mybir.AxisListType
OP = mybir.AluOpType
ACT = mybir.ActivationFunctionType

LAST_RUN = None

_CACHE = {}


def _tiles_c(ch):
    """Split ch chunks into NTILES chunk-aligned DMA tiles."""
    base, rem = divmod(ch, NTILES)
    return [base + 1] * rem + [base] * (NTILES - rem)


def _build(reps=1, hw_loop=1, mode="full", ch=CH):
    """reps: python-unrolled kernel bodies; hw_loop > 1 additionally wraps
    them in a hardware For_i loop (total bodies = reps * hw_loop) so timing
    NEFFs can amortize the ~100 ms (+/- tens of ms) axon dispatch jitter over
    hundreds of bodies without exploding the instruction count.

    mode: diagnostic bodies for attributing HW time -- "full" (the real
    kernel), "dma" (x DMA stream only), "pe" (matmuls+softmax from a static
    SBUF tile, no x DMAs), "both" (DMA stream + static-tile matmuls: no
    data dependency between the two streams)."""
    padc = 128 * ch
    w2 = 2 * ch
    assert w2 <= 512, "scores for one pair must fit a single PSUM bank"

    nc = bacc.Bacc(trn_type="TRN2", enable_partition_id=False,
                   num_devices=NCORES)
    xs = nc.dram_tensor("xs", [PAIRS, 128, padc], F16,
                        kind="ExternalInput").ap()
    amov_d = nc.dram_tensor("amov", [128, 2], F16, kind="ExternalInput").ap()
    tgtv_d = nc.dram_tensor("tgtv", [128, BPC], F32,
                            kind="ExternalInput").ap()
    attn = nc.dram_tensor("attn", [BPC, padc], F32, kind="ExternalOutput").ap()

    tiles_c = _tiles_c(ch)

    with tile.TileContext(nc) as tc, ExitStack() as ctx:
        singles = ctx.enter_context(tc.tile_pool(name="singles", bufs=1))
        xpool = ctx.enter_context(tc.tile_pool(name="xpool", bufs=XB))
        epool = ctx.enter_context(tc.tile_pool(name="epool", bufs=2))
        stat = ctx.enter_context(tc.tile_pool(name="stat", bufs=8))
        psco = ctx.enter_context(tc.tile_pool(name="psco", bufs=2,
                                              space="PSUM"))

        amov_sb = singles.tile([128, 2], F16)
        with tc.high_priority():
            nc.sync.dma_start(out=amov_sb, in_=amov_d)
        tgtv_sb = singles.tile([128, BPC], F32)
        nc.scalar.dma_start(out=tgtv_sb, in_=tgtv_d)
        state = {"first_tile": True}

        def body():
            for _ in range(reps):
                _one_rep()

        xstat = None
        if mode in ("pe", "both"):
            xstat = singles.tile([128, tiles_c[0] * 128], F16)
            nc.vector.memset(xstat, 0.0)

        def _one_rep():
            for j in range(PAIRS):
                # --- load xh tiles and run one matmul per 128-node chunk ---
                xts = []
                c0 = 0
                for t, tcn in enumerate(tiles_c):
                    f0, f1 = c0 * 128, (c0 + tcn) * 128
                    if mode == "pe":
                        c0 += tcn
                        continue
                    xt = xpool.tile([128, f1 - f0], F16)
                    if state["first_tile"]:
                        # quarter the very first DMA so PE starts ~4x earlier
                        state["first_tile"] = False
                        q = (f1 - f0) // 4
                        with tc.high_priority():
                            for i in range(4):
                                nc.sync.dma_start(
                                    out=xt[:, i * q:(i + 1) * q],
                                    in_=xs[j, :, f0 + i * q:f0 + (i + 1) * q])
                    else:
                        eng = nc.scalar if (RING_SPLIT and t % 2) else nc.sync
                        eng.dma_start(out=xt, in_=xs[j, :, f0:f1])
                    xts.append((c0, xt))
                    c0 += tcn
                if mode == "dma":
                    continue

                ps = psco.tile([128, 512], F32, tag="sc")
                for c in range(ch):
                    if mode in ("pe", "both"):
                        lhsT = xstat[:, (c % tiles_c[0]) * 128:
                                     (c % tiles_c[0]) * 128 + 128]
                    else:
                        tc0, xt = next((t0, x) for t0, x in reversed(xts)
                                       if t0 <= c)
                        lhsT = xt[:, (c - tc0) * 128:(c - tc0) * 128 + 128]
                    nc.tensor.matmul(ps[:, 2 * c:2 * c + 2], lhsT, amov_sb,
                                     start=True, stop=True)

                # --- softmax tail, straight out of PSUM ---
                # z = leaky_relu(scores + tgt, 0.01).  Scores are O(10): exp
                # cannot overflow fp32, so no max-subtraction.  Pad slots
                # carry score -30000 and vanish under exp.
                for bi in range(2):
                    b = 2 * j + bi
                    zb = epool.tile([128, ch], F32, tag=f"zb{bi}")
                    nc.vector.tensor_scalar_add(zb, ps[:, bi:w2:2],
                                                tgtv_sb[:, b:b + 1])
                    nc.vector.scalar_tensor_tensor(zb, zb, 0.01, zb,
                                                   op0=OP.mult, op1=OP.max)
                    # exp with per-partition row sums; global sum + broadcast
                    # in ONE idle-GPSIMD op (daisy-chain all-reduce).
                    pb = epool.tile([128, ch], F32, tag=f"pb{bi}")
                    srow = stat.tile([128, 1], F32)
                    nc.scalar.activation(pb, zb, ACT.Exp,
                                         bias=0.0, scale=1.0, accum_out=srow)
                    gsum = stat.tile([128, 1], F32, tag="gsum")
                    nc.gpsimd.partition_all_reduce(gsum, srow, 128,
                                                   bass_isa.ReduceOp.add)
                    rec = stat.tile([128, 1], F32, tag="rec")
                    nc.vector.reciprocal(rec, gsum)
                    nc.vector.tensor_scalar_mul(pb, pb, rec)
                    nc.scalar.dma_start(
                        out=attn[b].rearrange("(p c) -> p c", c=ch),
                        in_=pb)

        if hw_loop > 1:
            with tc.For_i(0, hw_loop):
                body()
        else:
            body()
    nc.compile()
    return nc


def _host_prep(x, a, node_index, adj_mask):
    x = np.asarray(x, dtype=np.float32)
    a = np.asarray(a, dtype=np.float32).reshape(2 * D)
    adj = np.asarray(adj_mask)
    idx = int(node_index)
    a_tgt, a_src = a[:D], a[D:]

    tgt = (x[:, idx, :] @ a_tgt).astype(np.float32)          # [B]

    keep = np.flatnonzero(adj > 0)                           # [M] node ids
    m = len(keep)
    assert m > 0, "all-masked adjacency not supported"
    ch = max(CH, -(-m // 128))                               # capacity chunks
    padc = 128 * ch

    # col = c*128 + p  <->  compact slot k = p*ch + c
    kk = (np.arange(ch)[:, None] + np.arange(128)[None, :] * ch).ravel()
    valid = kk < m

    # xh[pair, bi*64+d, col] = x[2*pair+bi, keep[kk[col]], d]  (fp16)
    xt16 = np.ascontiguousarray(x.transpose(0, 2, 1), dtype=np.float16)
    xh = xt16[:, :, keep[np.minimum(kk, m - 1)]]             # [B, D, padc]
    w = (a_src * (NEGW / max(float(a_src @ a_src), 1e-12))).astype(np.float16)
    xh[:, :, ~valid] = w[None, :, None]
    xh = np.ascontiguousarray(xh).reshape(B // 2, 128, padc)

    amov = np.zeros((128, 2), np.float16)
    amov[0:D, 0] = a_src
    amov[D:2 * D, 1] = a_src

    return xh, amov, tgt, keep, ch


def _in_maps(xh, amov, tgt, keep, ch):
    maps = []
    for c in range(NCORES):
        tgtv = np.ascontiguousarray(np.broadcast_to(
            tgt[c * BPC:(c + 1) * BPC][None, :], (128, BPC)).astype(
                np.float32))
        maps.append({
            "xs": xh[c * PAIRS:(c + 1) * PAIRS],
            "amov": amov,
            "tgtv": tgtv,
        })
    return maps


def _runner(ch=CH):
    """Build the Bass program once and wrap its NEFF custom call in a jitted
    shard_map over the 8 cores."""
    key = ("runner", ch)
    if key in _CACHE:
        return _CACHE[key]
    install_neuronx_cc_hook()
    nc = _CACHE.setdefault(("nc", ch), _build(ch=ch))
    in_names, out_names, out_avals, zero_shapes = [], [], [], []
    for alloc in nc.m.functions[0].allocations:
        if not isinstance(alloc, mybir.MemoryLocationSet):
            continue
        name = alloc.memorylocations[0].name
        if alloc.kind == "ExternalInput":
            in_names.append(name)
        elif alloc.kind == "ExternalOutput":
            out_names.append(name)
            shape = tuple(alloc.tensor_shape)
            dtype = mybir.dt.np(alloc.dtype)
            out_avals.append(jax.core.ShapedArray(shape, dtype))
            zero_shapes.append((shape, dtype))

    def _body(*args):
        return tuple(_bass_exec_p.bind(
            *args,
            out_avals=tuple(out_avals),
            in_names=tuple(in_names + out_names),
            out_names=tuple(out_names),
            lowering_input_output_aliases=(),
            sim_require_finite=True,
            sim_require_nnan=True,
            nc=nc,
        ))

    mesh = Mesh(np.asarray(jax.devices()[:NCORES]), ("core",))
    nin = len(in_names) + len(out_names)
    sharded = jax.jit(shard_map(
        _body, mesh=mesh,
        in_specs=(PartitionSpec("core"),) * nin,
        out_specs=(PartitionSpec("core"),) * len(out_names),
        check_rep=False))
    _CACHE[key] = (sharded, in_names, out_names, zero_shapes)
    return _CACHE[key]


def kernel(x, a, node_index, adj_mask):
    global LAST_RUN
    prep = _host_prep(x, a, node_index, adj_mask)
    ch, keep, m = prep[4], prep[3], len(prep[3])
    maps = _in_maps(*prep)
    sharded, in_names, out_names, zero_shapes = _runner(ch=ch)
    # concat of the 8 per-core xs shards is exactly the full array
    ins = [prep[0] if nm == "xs" else
           np.concatenate([mp[nm] for mp in maps], axis=0)
           for nm in in_names]
    zeros = [np.zeros((NCORES * s[0], *s[1:]), d) for s, d in zero_shapes]
    outs = sharded(*ins, *zeros)
    LAST_RUN = outs
    attn_c = np.asarray(outs[out_names.index("attn")])  # [B, 128*ch]
    full = np.zeros((B, N), np.float32)
    full[:, keep] = attn_c[:, :m]
    return full


# revision 10
# speedup vs baseline: 2.5414x; 1.0235x over previous
"""GNN attention (GAT-style single-target-node) kernel for 8 Trainium2 cores.

Problem:  x [32, 50000, 64], a [128, 1], node_index scalar, adj_mask [50000]
  tgt_score = x[:, idx] @ a[:64]                             # [B]
  e = leaky_relu(tgt_score[:, None] + x @ a[64:], 0.01)      # [B, N]
  attention = softmax(where(adj>0, e, -9e15), axis=1) * adj  # [B, N]

Key observation: rows with adj_mask == 0 contribute exp(-9e15) = 0 to the
softmax denominator and their output is *exactly* 0 (softmax * adj).  So the
device only ever needs x at the ~25k unmasked nodes: the host compacts
x[:, keep, :] (fp16) before upload and scatters the compact attention back
into a zero [B, N] canvas afterwards.  This halves HBM traffic vs the dense
kernel and removes the mask-bias grid entirely.

Sharding: data-parallel over batch (32 = 8 cores x 4 batches/core), the 4
batches paired into 2 batch-pairs.  Each core computes complete softmax rows,
so no cross-core reductions.

Per pair the host lays compact x out as xh[pair, bi*64+d, col] (fp16, both
batches of a pair stacked on the contraction axis).  Each 128-node chunk is
the *stationary* matmul operand [K=128, M=128 nodes] and the tiny constant
a-matrix [128, 2] streams as the moving operand, so LDWEIGHTS itself is the
data pass (128 values/cycle) and out = [128 nodes, 2 batches] lands
interleaved in ONE PSUM bank (2*CH <= 512).  Column permutation
col = c*128 + p <-> compact slot p*CH + c makes the final attention write
contiguous per partition.  Pad slots (>= M) carry a host-built vector w with
w @ a_src = -30000 so they vanish under exp() with no mask read or multiply.
The per-batch tgt_score is added as a [128,1] per-partition DVE scalar
operand (2 KB) instead of a broadcast [128, W] grid (800 KB).  Attention is
written back as fp16 scaled by 8192 (halves the output DMA; the host divides
the scale back out in fp32).

Measured per-body attribution (hw-loop steady state, 8 cores): x DMA stream
36.6 us (349 GB/s, at the ~360 GB/s per-core bus limit), PE score pass 17.4
us, full body 42.6 us -- DMA and PE SBUF traffic partially serialize on this
part, so the body floor is the DMA stream plus a ~5 us contention tax, not
max(DMA, PE)."""

import numpy as np
from contextlib import ExitStack

import jax
from jax.sharding import Mesh, PartitionSpec
from jax.experimental.shard_map import shard_map

import concourse.bass as bass
import concourse.bass_isa as bass_isa
import concourse.bacc as bacc
import concourse.tile as tile
from concourse import mybir
from concourse.bass2jax import _bass_exec_p, install_neuronx_cc_hook

B, N, D = 32, 50000, 64
NCORES = 8
BPC = B // NCORES            # batches per core
PAIRS = BPC // 2             # batch-pairs per core
CH = 195                     # 128-node chunks per batch: 128*195 = 24960 >= M
NTILES = 4                   # xh DMA tiles per pair
XB = 6                       # x-tile pool depth
RING_SPLIT = False           # alternate x-tile DMAs across sync/scalar HWDGE
NEGW = -30000.0              # pad-slot score (vanishes under exp after lrelu)

F32 = mybir.dt.float32
F16 = mybir.dt.float16
AX = mybir.AxisListType
OP = mybir.AluOpType
ACT = mybir.ActivationFunctionType

LAST_RUN = None

_CACHE = {}


def _tiles_c(ch, ntiles=NTILES):
    """Split ch chunks into ntiles chunk-aligned DMA tiles."""
    base, rem = divmod(ch, ntiles)
    return [base + 1] * rem + [base] * (ntiles - rem)


OUT16 = True                 # write attention as fp16 scaled by OUT_SCALE
OUT_SCALE = 8192.0


def _build(reps=1, hw_loop=1, mode="full", ch=CH, ntiles=NTILES, xb=XB,
           psb=2, out16=None, ring_split=None):
    """reps: python-unrolled kernel bodies; hw_loop > 1 additionally wraps
    them in a hardware For_i loop (total bodies = reps * hw_loop) so timing
    NEFFs can amortize the ~100 ms (+/- tens of ms) axon dispatch jitter over
    hundreds of bodies without exploding the instruction count.

    mode: diagnostic bodies for attributing HW time -- "full" (the real
    kernel), "dma" (x DMA stream only), "pe" (matmuls+softmax from a static
    SBUF tile, no x DMAs), "both" (DMA stream + static-tile matmuls: no
    data dependency between the two streams)."""
    if out16 is None:
        out16 = OUT16
    if ring_split is None:
        ring_split = RING_SPLIT
    padc = 128 * ch
    w2 = 2 * ch
    assert w2 <= 512, "scores for one pair must fit a single PSUM bank"

    nc = bacc.Bacc(trn_type="TRN2", enable_partition_id=False,
                   num_devices=NCORES)
    xs = nc.dram_tensor("xs", [PAIRS, 128, padc], F16,
                        kind="ExternalInput").ap()
    amov_d = nc.dram_tensor("amov", [128, 2], F16, kind="ExternalInput").ap()
    tgtv_d = nc.dram_tensor("tgtv", [128, BPC], F32,
                            kind="ExternalInput").ap()
    attn = nc.dram_tensor("attn", [BPC, padc], F16 if out16 else F32,
                          kind="ExternalOutput").ap()

    tiles_c = _tiles_c(ch, ntiles)

    with tile.TileContext(nc) as tc, ExitStack() as ctx:
        singles = ctx.enter_context(tc.tile_pool(name="singles", bufs=1))
        xpool = ctx.enter_context(tc.tile_pool(name="xpool", bufs=xb))
        epool = ctx.enter_context(tc.tile_pool(name="epool", bufs=2))
        stat = ctx.enter_context(tc.tile_pool(name="stat", bufs=8))
        psco = ctx.enter_context(tc.tile_pool(name="psco", bufs=psb,
                                              space="PSUM"))

        amov_sb = singles.tile([128, 2], F16)
        with tc.high_priority():
            nc.sync.dma_start(out=amov_sb, in_=amov_d)
        tgtv_sb = singles.tile([128, BPC], F32)
        nc.scalar.dma_start(out=tgtv_sb, in_=tgtv_d)
        state = {"first_tile": True}

        def body():
            for _ in range(reps):
                _one_rep()

        xstat = None
        if mode in ("pe", "both"):
            xstat = singles.tile([128, tiles_c[0] * 128], F16)
            nc.vector.memset(xstat, 0.0)

        def _one_rep():
            for j in range(PAIRS):
                # --- load xh tiles and run one matmul per 128-node chunk ---
                xts = []
                c0 = 0
                for t, tcn in enumerate(tiles_c):
                    f0, f1 = c0 * 128, (c0 + tcn) * 128
                    if mode == "pe":
                        c0 += tcn
                        continue
                    xt = xpool.tile([128, f1 - f0], F16)
                    if state["first_tile"]:
                        # quarter the very first DMA so PE starts ~4x earlier
                        state["first_tile"] = False
                        q = (f1 - f0) // 4
                        with tc.high_priority():
                            for i in range(4):
                                nc.sync.dma_start(
                                    out=xt[:, i * q:(i + 1) * q],
                                    in_=xs[j, :, f0 + i * q:f0 + (i + 1) * q])
                    else:
                        eng = nc.scalar if (ring_split and t % 2) else nc.sync
                        eng.dma_start(out=xt, in_=xs[j, :, f0:f1])
                    xts.append((c0, xt))
                    c0 += tcn
                if mode == "dma":
                    continue

                ps = psco.tile([128, 512], F32, tag="sc")
                for c in range(ch):
                    if mode in ("pe", "both"):
                        lhsT = xstat[:, (c % tiles_c[0]) * 128:
                                     (c % tiles_c[0]) * 128 + 128]
                    else:
                        tc0, xt = next((t0, x) for t0, x in reversed(xts)
                                       if t0 <= c)
                        lhsT = xt[:, (c - tc0) * 128:(c - tc0) * 128 + 128]
                    nc.tensor.matmul(ps[:, 2 * c:2 * c + 2], lhsT, amov_sb,
                                     start=True, stop=True)

                # --- softmax tail, straight out of PSUM ---
                # z = leaky_relu(scores + tgt, 0.01).  Scores are O(10): exp
                # cannot overflow fp32, so no max-subtraction.  Pad slots
                # carry score -30000 and vanish under exp.
                for bi in range(2):
                    b = 2 * j + bi
                    zb = epool.tile([128, ch], F32, tag=f"zb{bi}")
                    nc.vector.tensor_scalar_add(zb, ps[:, bi:w2:2],
                                                tgtv_sb[:, b:b + 1])
                    nc.vector.scalar_tensor_tensor(zb, zb, 0.01, zb,
                                                   op0=OP.mult, op1=OP.max)
                    # exp with per-partition row sums; global sum + broadcast
                    # in ONE idle-GPSIMD op (daisy-chain all-reduce).
                    pb = epool.tile([128, ch], F32, tag=f"pb{bi}")
                    srow = stat.tile([128, 1], F32)
                    nc.scalar.activation(pb, zb, ACT.Exp,
                                         bias=0.0, scale=1.0, accum_out=srow)
                    gsum = stat.tile([128, 1], F32, tag="gsum")
                    nc.gpsimd.partition_all_reduce(gsum, srow, 128,
                                                   bass_isa.ReduceOp.add)
                    rec = stat.tile([128, 1], F32, tag="rec")
                    nc.vector.reciprocal(rec, gsum)
                    if out16:
                        # x OUT_SCALE keeps fp16 out of the subnormal range
                        # (attention ~ 1/25000); host divides it back out.
                        po = epool.tile([128, ch], F16, tag=f"po{bi}")
                        nc.vector.tensor_scalar(po, pb, rec, OUT_SCALE,
                                                op0=OP.mult, op1=OP.mult)
                    else:
                        po = pb
                        nc.vector.tensor_scalar_mul(pb, pb, rec)
                    nc.scalar.dma_start(
                        out=attn[b].rearrange("(p c) -> p c", c=ch),
                        in_=po)

        if hw_loop > 1:
            with tc.For_i(0, hw_loop):
                body()
        else:
            body()
    nc.compile()
    return nc


def _host_prep(x, a, node_index, adj_mask):
    x = np.asarray(x, dtype=np.float32)
    a = np.asarray(a, dtype=np.float32).reshape(2 * D)
    adj = np.asarray(adj_mask)
    idx = int(node_index)
    a_tgt, a_src = a[:D], a[D:]

    tgt = (x[:, idx, :] @ a_tgt).astype(np.float32)          # [B]

    keep = np.flatnonzero(adj > 0)                           # [M] node ids
    m = len(keep)
    assert m > 0, "all-masked adjacency not supported"
    ch = max(CH, -(-m // 128))                               # capacity chunks
    padc = 128 * ch

    # col = c*128 + p  <->  compact slot k = p*ch + c
    kk = (np.arange(ch)[:, None] + np.arange(128)[None, :] * ch).ravel()
    valid = kk < m

    # xh[pair, bi*64+d, col] = x[2*pair+bi, keep[kk[col]], d]  (fp16)
    xt16 = np.ascontiguousarray(x.transpose(0, 2, 1), dtype=np.float16)
    xh = xt16[:, :, keep[np.minimum(kk, m - 1)]]             # [B, D, padc]
    w = (a_src * (NEGW / max(float(a_src @ a_src), 1e-12))).astype(np.float16)
    xh[:, :, ~valid] = w[None, :, None]
    xh = np.ascontiguousarray(xh).reshape(B // 2, 128, padc)

    amov = np.zeros((128, 2), np.float16)
    amov[0:D, 0] = a_src
    amov[D:2 * D, 1] = a_src

    return xh, amov, tgt, keep, ch


def _in_maps(xh, amov, tgt, keep, ch):
    maps = []
    for c in range(NCORES):
        tgtv = np.ascontiguousarray(np.broadcast_to(
            tgt[c * BPC:(c + 1) * BPC][None, :], (128, BPC)).astype(
                np.float32))
        maps.append({
            "xs": xh[c * PAIRS:(c + 1) * PAIRS],
            "amov": amov,
            "tgtv": tgtv,
        })
    return maps


def _runner(ch=CH):
    """Build the Bass program once and wrap its NEFF custom call in a jitted
    shard_map over the 8 cores."""
    key = ("runner", ch)
    if key in _CACHE:
        return _CACHE[key]
    install_neuronx_cc_hook()
    nc = _CACHE.setdefault(("nc", ch), _build(ch=ch))
    in_names, out_names, out_avals, zero_shapes = [], [], [], []
    for alloc in nc.m.functions[0].allocations:
        if not isinstance(alloc, mybir.MemoryLocationSet):
            continue
        name = alloc.memorylocations[0].name
        if alloc.kind == "ExternalInput":
            in_names.append(name)
        elif alloc.kind == "ExternalOutput":
            out_names.append(name)
            shape = tuple(alloc.tensor_shape)
            dtype = mybir.dt.np(alloc.dtype)
            out_avals.append(jax.core.ShapedArray(shape, dtype))
            zero_shapes.append((shape, dtype))

    def _body(*args):
        return tuple(_bass_exec_p.bind(
            *args,
            out_avals=tuple(out_avals),
            in_names=tuple(in_names + out_names),
            out_names=tuple(out_names),
            lowering_input_output_aliases=(),
            sim_require_finite=True,
            sim_require_nnan=True,
            nc=nc,
        ))

    mesh = Mesh(np.asarray(jax.devices()[:NCORES]), ("core",))
    nin = len(in_names) + len(out_names)
    sharded = jax.jit(shard_map(
        _body, mesh=mesh,
        in_specs=(PartitionSpec("core"),) * nin,
        out_specs=(PartitionSpec("core"),) * len(out_names),
        check_rep=False))
    _CACHE[key] = (sharded, in_names, out_names, zero_shapes)
    return _CACHE[key]


def kernel(x, a, node_index, adj_mask):
    global LAST_RUN
    prep = _host_prep(x, a, node_index, adj_mask)
    ch, keep, m = prep[4], prep[3], len(prep[3])
    maps = _in_maps(*prep)
    sharded, in_names, out_names, zero_shapes = _runner(ch=ch)
    # concat of the 8 per-core xs shards is exactly the full array
    ins = [prep[0] if nm == "xs" else
           np.concatenate([mp[nm] for mp in maps], axis=0)
           for nm in in_names]
    zeros = [np.zeros((NCORES * s[0], *s[1:]), d) for s, d in zero_shapes]
    outs = sharded(*ins, *zeros)
    LAST_RUN = outs
    attn_c = np.asarray(outs[out_names.index("attn")])  # [B, 128*ch]
    full = np.zeros((B, N), np.float32)
    if attn_c.dtype == np.float16:
        full[:, keep] = attn_c[:, :m].astype(np.float32) * (1.0 / OUT_SCALE)
    else:
        full[:, keep] = attn_c[:, :m]
    return full


# revision 21
# speedup vs baseline: 2.5601x; 1.0074x over previous
"""GNN attention (GAT-style single-target-node) kernel for 8 Trainium2 cores.

Problem:  x [32, 50000, 64], a [128, 1], node_index scalar, adj_mask [50000]
  tgt_score = x[:, idx] @ a[:64]                             # [B]
  e = leaky_relu(tgt_score[:, None] + x @ a[64:], 0.01)      # [B, N]
  attention = softmax(where(adj>0, e, -9e15), axis=1) * adj  # [B, N]

Key observation: rows with adj_mask == 0 contribute exp(-9e15) = 0 to the
softmax denominator and their output is *exactly* 0 (softmax * adj).  So the
device only ever needs x at the ~25k unmasked nodes: the host compacts
x[:, keep, :] (fp16) before upload and scatters the compact attention back
into a zero [B, N] canvas afterwards.  This halves HBM traffic vs the dense
kernel and removes the mask-bias grid entirely.

Sharding: data-parallel over batch (32 = 8 cores x 4 batches/core), the 4
batches paired into 2 batch-pairs.  Each core computes complete softmax rows,
so no cross-core reductions.

Per pair the host lays compact x out as xh[pair, bi*64+d, col] (fp16, both
batches of a pair stacked on the contraction axis).  Each 128-node chunk is
the *stationary* matmul operand [K=128, M=128 nodes] and the tiny constant
a-matrix [128, 2] streams as the moving operand, so LDWEIGHTS itself is the
data pass (128 values/cycle) and out = [128 nodes, 2 batches] lands
interleaved in PSUM (one bank per 256 chunks; a single bank for the seed-0
mask, where ch = 195 <= 256).  Column permutation
col = c*128 + p <-> compact slot p*CH + c makes the final attention write
contiguous per partition.  Pad slots (>= M) carry a host-built vector w with
w @ a_src = -30000 so they vanish under exp() with no mask read or multiply.
The per-batch tgt_score is added as a [128,1] per-partition DVE scalar
operand (2 KB) instead of a broadcast [128, W] grid (800 KB).  Attention is
written back as fp16 scaled by 8192 (halves the output DMA; the host divides
the scale back out in fp32).

Measured per-body attribution (hw-loop steady state, 8 cores): x DMA stream
36.6 us (349 GB/s, at the ~360 GB/s per-core bus limit), PE score pass 17.4
us, full body 42.6 us -- DMA and PE SBUF traffic partially serialize on this
part, so the body floor is the DMA stream plus a ~5 us contention tax, not
max(DMA, PE)."""

import numpy as np
from contextlib import ExitStack

import jax
from jax.sharding import Mesh, PartitionSpec
from jax.experimental.shard_map import shard_map

import concourse.bass as bass
import concourse.bass_isa as bass_isa
import concourse.bacc as bacc
import concourse.tile as tile
from concourse import mybir
from concourse.bass2jax import _bass_exec_p, install_neuronx_cc_hook

B, N, D = 32, 50000, 64
NCORES = 8
BPC = B // NCORES            # batches per core
PAIRS = BPC // 2             # batch-pairs per core
CH = 195                     # floor on 128-node chunks; actual ch adapts to M
NTILES = 4                   # xh DMA tiles per pair
XB = 6                       # x-tile pool depth
RING_SPLIT = False           # alternate x-tile DMAs across sync/scalar HWDGE
NEGW = -30000.0              # pad-slot score (vanishes under exp after lrelu)

F32 = mybir.dt.float32
F16 = mybir.dt.float16
AX = mybir.AxisListType
OP = mybir.AluOpType
ACT = mybir.ActivationFunctionType

LAST_RUN = None

_CACHE = {}


def _tiles_c(ch, ntiles=NTILES):
    """Split ch chunks into ntiles chunk-aligned DMA tiles."""
    base, rem = divmod(ch, ntiles)
    return [base + 1] * rem + [base] * (ntiles - rem)


OUT16 = True                 # write attention as fp16 scaled by OUT_SCALE
OUT_SCALE = 8192.0


ACT_LRELU = False            # fuse add+leaky into one Act-engine Lrelu op


def _build(reps=1, hw_loop=1, mode="full", ch=CH, ntiles=NTILES, xb=XB,
           psb=2, out16=None, ring_split=None, act_lrelu=None):
    """reps: python-unrolled kernel bodies; hw_loop > 1 additionally wraps
    them in a hardware For_i loop (total bodies = reps * hw_loop) so timing
    NEFFs can amortize the ~100 ms (+/- tens of ms) axon dispatch jitter over
    hundreds of bodies without exploding the instruction count.

    mode: diagnostic bodies for attributing HW time -- "full" (the real
    kernel), "dma" (x DMA stream only), "pe" (matmuls+softmax from a static
    SBUF tile, no x DMAs), "both" (DMA stream + static-tile matmuls: no
    data dependency between the two streams)."""
    if out16 is None:
        out16 = OUT16
    if ring_split is None:
        ring_split = RING_SPLIT
    if act_lrelu is None:
        act_lrelu = ACT_LRELU
    padc = 128 * ch
    cpb = 256                # chunks per PSUM bank (512 f32 cols)
    nbank = -(-ch // cpb)    # PSUM banks per pair (1 for ch <= 256)
    assert psb * nbank <= 8, "PSUM overflow: mask too dense for this config"

    nc = bacc.Bacc(trn_type="TRN2", enable_partition_id=False,
                   num_devices=NCORES)
    xs = nc.dram_tensor("xs", [PAIRS, 128, padc], F16,
                        kind="ExternalInput").ap()
    amov_d = nc.dram_tensor("amov", [128, 2], F16, kind="ExternalInput").ap()
    tgtv_d = nc.dram_tensor("tgtv", [128, BPC], F32,
                            kind="ExternalInput").ap()
    attn = nc.dram_tensor("attn", [BPC, padc], F16 if out16 else F32,
                          kind="ExternalOutput").ap()

    tiles_c = _tiles_c(ch, ntiles)

    with tile.TileContext(nc) as tc, ExitStack() as ctx:
        singles = ctx.enter_context(tc.tile_pool(name="singles", bufs=1))
        xpool = ctx.enter_context(tc.tile_pool(name="xpool", bufs=xb))
        epool = ctx.enter_context(tc.tile_pool(name="epool", bufs=2))
        stat = ctx.enter_context(tc.tile_pool(name="stat", bufs=8))
        psco = ctx.enter_context(tc.tile_pool(name="psco", bufs=psb,
                                              space="PSUM"))

        amov_sb = singles.tile([128, 2], F16)
        with tc.high_priority():
            nc.sync.dma_start(out=amov_sb, in_=amov_d)
        tgtv_sb = singles.tile([128, BPC], F32)
        nc.scalar.dma_start(out=tgtv_sb, in_=tgtv_d)
        state = {"first_tile": True}

        def body():
            for _ in range(reps):
                _one_rep()

        xstat = None
        if mode in ("pe", "both"):
            xstat = singles.tile([128, tiles_c[0] * 128], F16)
            nc.vector.memset(xstat, 0.0)

        def _one_rep():
            for j in range(PAIRS):
                # --- load xh tiles and run one matmul per 128-node chunk ---
                xts = []
                c0 = 0
                for t, tcn in enumerate(tiles_c):
                    f0, f1 = c0 * 128, (c0 + tcn) * 128
                    if mode == "pe":
                        c0 += tcn
                        continue
                    xt = xpool.tile([128, f1 - f0], F16)
                    if state["first_tile"]:
                        # quarter the very first DMA so PE starts ~4x earlier
                        state["first_tile"] = False
                        q = (f1 - f0) // 4
                        with tc.high_priority():
                            for i in range(4):
                                nc.sync.dma_start(
                                    out=xt[:, i * q:(i + 1) * q],
                                    in_=xs[j, :, f0 + i * q:f0 + (i + 1) * q])
                    else:
                        eng = nc.scalar if (ring_split and t % 2) else nc.sync
                        eng.dma_start(out=xt, in_=xs[j, :, f0:f1])
                    xts.append((c0, xt))
                    c0 += tcn
                if mode == "dma":
                    continue

                ps = [psco.tile([128, 512], F32, tag=f"sc{k}",
                                name=f"ps{k}")
                      for k in range(nbank)]
                for c in range(ch):
                    if mode in ("pe", "both"):
                        lhsT = xstat[:, (c % tiles_c[0]) * 128:
                                     (c % tiles_c[0]) * 128 + 128]
                    else:
                        tc0, xt = next((t0, x) for t0, x in reversed(xts)
                                       if t0 <= c)
                        lhsT = xt[:, (c - tc0) * 128:(c - tc0) * 128 + 128]
                    bk, cb = divmod(c, cpb)
                    nc.tensor.matmul(ps[bk][:, 2 * cb:2 * cb + 2], lhsT,
                                     amov_sb, start=True, stop=True)

                # --- softmax tail, straight out of PSUM ---
                # z = leaky_relu(scores + tgt, 0.01).  Scores are O(10): exp
                # cannot overflow fp32, so no max-subtraction.  Pad slots
                # carry score -30000 and vanish under exp.
                for bi in range(2):
                    b = 2 * j + bi
                    zb = epool.tile([128, ch], F32, tag=f"zb{bi}")
                    if act_lrelu:
                        # zb = lrelu(ps + tgt, 0.01) in one Act-engine op
                        for k in range(nbank):
                            c1 = min(ch, (k + 1) * cpb)
                            nc.scalar.activation(
                                zb[:, k * cpb:c1],
                                ps[k][:, bi:2 * (c1 - k * cpb):2],
                                ACT.Lrelu, bias=tgtv_sb[:, b:b + 1],
                                scale=1.0, alpha=0.01)
                    else:
                        for k in range(nbank):
                            c1 = min(ch, (k + 1) * cpb)
                            nc.vector.tensor_scalar_add(
                                zb[:, k * cpb:c1],
                                ps[k][:, bi:2 * (c1 - k * cpb):2],
                                tgtv_sb[:, b:b + 1])
                        nc.vector.scalar_tensor_tensor(zb, zb, 0.01, zb,
                                                       op0=OP.mult,
                                                       op1=OP.max)
                    # exp with per-partition row sums; global sum + broadcast
                    # in ONE idle-GPSIMD op (daisy-chain all-reduce).
                    pb = epool.tile([128, ch], F32, tag=f"pb{bi}")
                    srow = stat.tile([128, 1], F32)
                    nc.scalar.activation(pb, zb, ACT.Exp,
                                         bias=0.0, scale=1.0, accum_out=srow)
                    gsum = stat.tile([128, 1], F32, tag="gsum")
                    nc.gpsimd.partition_all_reduce(gsum, srow, 128,
                                                   bass_isa.ReduceOp.add)
                    rec = stat.tile([128, 1], F32, tag="rec")
                    nc.vector.reciprocal(rec, gsum)
                    if out16:
                        # x OUT_SCALE keeps fp16 out of the subnormal range
                        # (attention ~ 1/25000); host divides it back out.
                        po = epool.tile([128, ch], F16, tag=f"po{bi}")
                        nc.vector.tensor_scalar(po, pb, rec, OUT_SCALE,
                                                op0=OP.mult, op1=OP.mult)
                    else:
                        po = pb
                        nc.vector.tensor_scalar_mul(pb, pb, rec)
                    nc.scalar.dma_start(
                        out=attn[b].rearrange("(p c) -> p c", c=ch),
                        in_=po)

        if hw_loop > 1:
            with tc.For_i(0, hw_loop):
                body()
        else:
            body()
    nc.compile()
    return nc


def _host_prep(x, a, node_index, adj_mask):
    x = np.asarray(x, dtype=np.float32)
    a = np.asarray(a, dtype=np.float32).reshape(2 * D)
    adj = np.asarray(adj_mask)
    idx = int(node_index)
    a_tgt, a_src = a[:D], a[D:]

    tgt = (x[:, idx, :] @ a_tgt).astype(np.float32)          # [B]

    keep = np.flatnonzero(adj > 0)                           # [M] node ids
    m = len(keep)
    assert m > 0, "all-masked adjacency not supported"
    ch = max(CH, -(-m // 128))                               # capacity chunks
    padc = 128 * ch

    # col = c*128 + p  <->  compact slot k = p*ch + c
    kk = (np.arange(ch)[:, None] + np.arange(128)[None, :] * ch).ravel()
    valid = kk < m

    # xh[pair, bi*64+d, col] = x[2*pair+bi, keep[kk[col]], d]  (fp16)
    xt16 = np.ascontiguousarray(x.transpose(0, 2, 1), dtype=np.float16)
    xh = xt16[:, :, keep[np.minimum(kk, m - 1)]]             # [B, D, padc]
    w = (a_src * (NEGW / max(float(a_src @ a_src), 1e-12))).astype(np.float16)
    xh[:, :, ~valid] = w[None, :, None]
    xh = np.ascontiguousarray(xh).reshape(B // 2, 128, padc)

    amov = np.zeros((128, 2), np.float16)
    amov[0:D, 0] = a_src
    amov[D:2 * D, 1] = a_src

    return xh, amov, tgt, keep, ch


def _in_maps(xh, amov, tgt, keep, ch):
    maps = []
    for c in range(NCORES):
        tgtv = np.ascontiguousarray(np.broadcast_to(
            tgt[c * BPC:(c + 1) * BPC][None, :], (128, BPC)).astype(
                np.float32))
        maps.append({
            "xs": xh[c * PAIRS:(c + 1) * PAIRS],
            "amov": amov,
            "tgtv": tgtv,
        })
    return maps


def _runner(ch=CH):
    """Build the Bass program once and wrap its NEFF custom call in a jitted
    shard_map over the 8 cores."""
    key = ("runner", ch)
    if key in _CACHE:
        return _CACHE[key]
    install_neuronx_cc_hook()
    nc = _CACHE.setdefault(("nc", ch), _build(ch=ch))
    in_names, out_names, out_avals, zero_shapes = [], [], [], []
    for alloc in nc.m.functions[0].allocations:
        if not isinstance(alloc, mybir.MemoryLocationSet):
            continue
        name = alloc.memorylocations[0].name
        if alloc.kind == "ExternalInput":
            in_names.append(name)
        elif alloc.kind == "ExternalOutput":
            out_names.append(name)
            shape = tuple(alloc.tensor_shape)
            dtype = mybir.dt.np(alloc.dtype)
            out_avals.append(jax.core.ShapedArray(shape, dtype))
            zero_shapes.append((shape, dtype))

    def _body(*args):
        return tuple(_bass_exec_p.bind(
            *args,
            out_avals=tuple(out_avals),
            in_names=tuple(in_names + out_names),
            out_names=tuple(out_names),
            lowering_input_output_aliases=(),
            sim_require_finite=True,
            sim_require_nnan=True,
            nc=nc,
        ))

    mesh = Mesh(np.asarray(jax.devices()[:NCORES]), ("core",))
    nin = len(in_names) + len(out_names)
    sharded = jax.jit(shard_map(
        _body, mesh=mesh,
        in_specs=(PartitionSpec("core"),) * nin,
        out_specs=(PartitionSpec("core"),) * len(out_names),
        check_rep=False))
    _CACHE[key] = (sharded, in_names, out_names, zero_shapes)
    return _CACHE[key]


def kernel(x, a, node_index, adj_mask):
    global LAST_RUN
    prep = _host_prep(x, a, node_index, adj_mask)
    ch, keep, m = prep[4], prep[3], len(prep[3])
    maps = _in_maps(*prep)
    sharded, in_names, out_names, zero_shapes = _runner(ch=ch)
    # concat of the 8 per-core xs shards is exactly the full array
    ins = [prep[0] if nm == "xs" else
           np.concatenate([mp[nm] for mp in maps], axis=0)
           for nm in in_names]
    zeros = [np.zeros((NCORES * s[0], *s[1:]), d) for s, d in zero_shapes]
    outs = sharded(*ins, *zeros)
    LAST_RUN = outs
    attn_c = np.asarray(outs[out_names.index("attn")])  # [B, 128*ch]
    full = np.zeros((B, N), np.float32)
    if attn_c.dtype == np.float16:
        full[:, keep] = attn_c[:, :m].astype(np.float32) * (1.0 / OUT_SCALE)
    else:
        full[:, keep] = attn_c[:, :m]
    return full


# revision 29
# speedup vs baseline: 2.6062x; 1.0180x over previous
"""GNN attention (GAT-style single-target-node) kernel for 8 Trainium2 cores.

Problem:  x [32, 50000, 64], a [128, 1], node_index scalar, adj_mask [50000]
  tgt_score = x[:, idx] @ a[:64]                             # [B]
  e = leaky_relu(tgt_score[:, None] + x @ a[64:], 0.01)      # [B, N]
  attention = softmax(where(adj>0, e, -9e15), axis=1) * adj  # [B, N]

Key observation: rows with adj_mask == 0 contribute exp(-9e15) = 0 to the
softmax denominator and their output is *exactly* 0 (softmax * adj).  So the
device only ever needs x at the ~25k unmasked nodes: the host compacts
x[:, keep, :] (fp16) before upload and scatters the compact attention back
into a zero [B, N] canvas afterwards.  This halves HBM traffic vs the dense
kernel and removes the mask-bias grid entirely.

Sharding: data-parallel over batch (32 = 8 cores x 4 batches/core), the 4
batches paired into 2 batch-pairs.  Each core computes complete softmax rows,
so no cross-core reductions.

Per pair the host lays compact x out as xh[pair, bi*64+d, col] (fp16, both
batches of a pair stacked on the contraction axis).  Each 128-node chunk is
the *stationary* matmul operand [K=128, M=128 nodes] and the tiny constant
a-matrix [128, 2] streams as the moving operand, so LDWEIGHTS itself is the
data pass (128 values/cycle) and out = [128 nodes, 2 batches] lands
interleaved in PSUM (one bank per 256 chunks; a single bank for the seed-0
mask, where ch = 195 <= 256).  Column permutation
col = c*128 + p <-> compact slot p*CH + c makes the final attention write
contiguous per partition.  Pad slots (>= M) carry a host-built vector w with
w @ a_src = -30000 so they vanish under exp() with no mask read or multiply.
The per-batch tgt_score is added as a [128,1] per-partition DVE scalar
operand (2 KB) instead of a broadcast [128, W] grid (800 KB).  Attention is
written back as fp16 scaled by 8192 (halves the output DMA; the host divides
the scale back out in fp32).

Measured per-body attribution (hw-loop steady state, 8 cores): x DMA stream
36.6 us (349 GB/s, at the ~360 GB/s per-core bus limit), PE score pass 17.4
us, full body 42.6 us -- DMA and PE SBUF traffic partially serialize on this
part, so the body floor is the DMA stream plus a ~5 us contention tax, not
max(DMA, PE)."""

import numpy as np
from contextlib import ExitStack

import jax
from jax.sharding import Mesh, PartitionSpec
from jax.experimental.shard_map import shard_map

import concourse.bass as bass
import concourse.bass_isa as bass_isa
import concourse.bacc as bacc
import concourse.tile as tile
from concourse import mybir
from concourse.bass2jax import _bass_exec_p, install_neuronx_cc_hook

B, N, D = 32, 50000, 64
NCORES = 8
BPC = B // NCORES            # batches per core
PAIRS = BPC // 2             # batch-pairs per core
CH = 195                     # floor on 128-node chunks; actual ch adapts to M
NTILES = 4                   # xh DMA tiles per pair
XB = 6                       # x-tile pool depth
RING_SPLIT = False           # alternate x-tile DMAs across sync/scalar HWDGE
NEGW = -30000.0              # pad-slot score (vanishes under exp after lrelu)

F32 = mybir.dt.float32
F16 = mybir.dt.float16
AX = mybir.AxisListType
OP = mybir.AluOpType
ACT = mybir.ActivationFunctionType

LAST_RUN = None

_CACHE = {}


def _tiles_c(ch, ntiles=NTILES):
    """Split ch chunks into ntiles chunk-aligned DMA tiles."""
    base, rem = divmod(ch, ntiles)
    return [base + 1] * rem + [base] * (ntiles - rem)


OUT16 = True                 # write attention as fp16 scaled by OUT_SCALE
OUT_SCALE = 8192.0


ACT_LRELU = False            # fuse add+leaky into one Act-engine Lrelu op
# 0: zb/pb in SBUF.  1: pb (exp output) in PSUM.  2: full-PSUM tail --
# Act-engine Lrelu(ps+tgt) -> zb PSUM -> Exp -> pb PSUM (only po in SBUF).
# HW rule NCC_IBVF027: an instruction may read at most ONE input from PSUM.
TAIL_PSUM = 0


def _build(reps=1, hw_loop=1, mode="full", ch=CH, ntiles=NTILES, xb=XB,
           psb=2, out16=None, ring_split=None, act_lrelu=None,
           tail_psum=None):
    """reps: python-unrolled kernel bodies; hw_loop > 1 additionally wraps
    them in a hardware For_i loop (total bodies = reps * hw_loop) so timing
    NEFFs can amortize the ~100 ms (+/- tens of ms) axon dispatch jitter over
    hundreds of bodies without exploding the instruction count.

    mode: diagnostic bodies for attributing HW time -- "full" (the real
    kernel), "dma" (x DMA stream only), "pe" (matmuls+softmax from a static
    SBUF tile, no x DMAs), "both" (DMA stream + static-tile matmuls: no
    data dependency between the two streams)."""
    if out16 is None:
        out16 = OUT16
    if ring_split is None:
        ring_split = RING_SPLIT
    if act_lrelu is None:
        act_lrelu = ACT_LRELU
    if tail_psum is None:
        tail_psum = TAIL_PSUM
    padc = 128 * ch
    cpb = 256                # chunks per PSUM bank (512 f32 cols)
    nbank = -(-ch // cpb)    # PSUM banks per pair (1 for ch <= 256)
    if tail_psum and 2 * ch > 512:
        tail_psum = 0        # zb/pb packing needs one bank per 2*ch floats
    nb_tail = {0: 0, 1: 1, 2: 2}[tail_psum]
    assert psb * (nbank + nb_tail) <= 8, "PSUM overflow: mask too dense"

    nc = bacc.Bacc(trn_type="TRN2", enable_partition_id=False,
                   num_devices=NCORES)
    xs = nc.dram_tensor("xs", [PAIRS, 128, padc], F16,
                        kind="ExternalInput").ap()
    amov_d = nc.dram_tensor("amov", [128, 2], F16, kind="ExternalInput").ap()
    tgtv_d = nc.dram_tensor("tgtv", [128, BPC], F32,
                            kind="ExternalInput").ap()
    attn = nc.dram_tensor("attn", [BPC, padc], F16 if out16 else F32,
                          kind="ExternalOutput").ap()

    tiles_c = _tiles_c(ch, ntiles)

    with tile.TileContext(nc) as tc, ExitStack() as ctx:
        singles = ctx.enter_context(tc.tile_pool(name="singles", bufs=1))
        xpool = ctx.enter_context(tc.tile_pool(name="xpool", bufs=xb))
        epool = ctx.enter_context(tc.tile_pool(name="epool", bufs=2))
        stat = ctx.enter_context(tc.tile_pool(name="stat", bufs=8))
        psco = ctx.enter_context(tc.tile_pool(name="psco", bufs=psb,
                                              space="PSUM"))

        amov_sb = singles.tile([128, 2], F16)
        with tc.high_priority():
            nc.sync.dma_start(out=amov_sb, in_=amov_d)
        tgtv_sb = singles.tile([128, BPC], F32)
        nc.scalar.dma_start(out=tgtv_sb, in_=tgtv_d)
        state = {"first_tile": True}

        def body():
            for _ in range(reps):
                _one_rep()

        xstat = None
        if mode in ("pe", "both"):
            xstat = singles.tile([128, tiles_c[0] * 128], F16)
            nc.vector.memset(xstat, 0.0)

        def _one_rep():
            for j in range(PAIRS):
                # --- load xh tiles and run one matmul per 128-node chunk ---
                xts = []
                c0 = 0
                for t, tcn in enumerate(tiles_c):
                    f0, f1 = c0 * 128, (c0 + tcn) * 128
                    if mode == "pe":
                        c0 += tcn
                        continue
                    xt = xpool.tile([128, f1 - f0], F16)
                    if state["first_tile"]:
                        # quarter the very first DMA so PE starts ~4x earlier
                        state["first_tile"] = False
                        q = (f1 - f0) // 4
                        with tc.high_priority():
                            for i in range(4):
                                nc.sync.dma_start(
                                    out=xt[:, i * q:(i + 1) * q],
                                    in_=xs[j, :, f0 + i * q:f0 + (i + 1) * q])
                    else:
                        eng = nc.scalar if (ring_split and t % 2) else nc.sync
                        eng.dma_start(out=xt, in_=xs[j, :, f0:f1])
                    xts.append((c0, xt))
                    c0 += tcn
                if mode == "dma":
                    continue

                ps = [psco.tile([128, 512], F32, tag=f"sc{k}",
                                name=f"ps{k}")
                      for k in range(nbank)]
                for c in range(ch):
                    if mode in ("pe", "both"):
                        lhsT = xstat[:, (c % tiles_c[0]) * 128:
                                     (c % tiles_c[0]) * 128 + 128]
                    else:
                        tc0, xt = next((t0, x) for t0, x in reversed(xts)
                                       if t0 <= c)
                        lhsT = xt[:, (c - tc0) * 128:(c - tc0) * 128 + 128]
                    bk, cb = divmod(c, cpb)
                    nc.tensor.matmul(ps[bk][:, 2 * cb:2 * cb + 2], lhsT,
                                     amov_sb, start=True, stop=True)

                # --- softmax tail, straight out of PSUM ---
                # z = leaky_relu(scores + tgt, 0.01).  Scores are O(10): exp
                # cannot overflow fp32, so no max-subtraction.  Pad slots
                # carry score -30000 and vanish under exp.
                if tail_psum == 2:
                    # both parities packed into one PSUM bank per stage so
                    # tail intermediates never touch SBUF bandwidth
                    zt = psco.tile([128, 2 * ch], F32, tag="zt", name="zt")
                if tail_psum:
                    pt = psco.tile([128, 2 * ch], F32, tag="pt", name="pt")
                for bi in range(2):
                    b = 2 * j + bi
                    if tail_psum == 2:
                        zb = zt[:, bi * ch:(bi + 1) * ch]
                    else:
                        zb = epool.tile([128, ch], F32, tag=f"zb{bi}",
                                        name="zb")
                    if act_lrelu or tail_psum == 2:
                        # zb = lrelu(ps + tgt, 0.01) in one Act-engine op;
                        # single PSUM input per instruction (NCC_IBVF027)
                        for k in range(nbank):
                            c1 = min(ch, (k + 1) * cpb)
                            nc.scalar.activation(
                                zb[:, k * cpb:c1],
                                ps[k][:, bi:2 * (c1 - k * cpb):2],
                                ACT.Lrelu, bias=tgtv_sb[:, b:b + 1],
                                scale=1.0, alpha=0.01)
                    else:
                        for k in range(nbank):
                            c1 = min(ch, (k + 1) * cpb)
                            nc.vector.tensor_scalar_add(
                                zb[:, k * cpb:c1],
                                ps[k][:, bi:2 * (c1 - k * cpb):2],
                                tgtv_sb[:, b:b + 1])
                        nc.vector.scalar_tensor_tensor(zb, zb, 0.01, zb,
                                                       op0=OP.mult,
                                                       op1=OP.max)
                    # exp with per-partition row sums; global sum + broadcast
                    # in ONE idle-GPSIMD op (daisy-chain all-reduce).
                    if tail_psum:
                        pb = pt[:, bi * ch:(bi + 1) * ch]
                    else:
                        pb = epool.tile([128, ch], F32, tag=f"pb{bi}",
                                        name="pb")
                    srow = stat.tile([128, 1], F32)
                    nc.scalar.activation(pb, zb, ACT.Exp,
                                         bias=0.0, scale=1.0, accum_out=srow)
                    gsum = stat.tile([128, 1], F32, tag="gsum")
                    nc.gpsimd.partition_all_reduce(gsum, srow, 128,
                                                   bass_isa.ReduceOp.add)
                    rec = stat.tile([128, 1], F32, tag="rec")
                    nc.vector.reciprocal(rec, gsum)
                    if out16:
                        # x OUT_SCALE keeps fp16 out of the subnormal range
                        # (attention ~ 1/25000); host divides it back out.
                        po = epool.tile([128, ch], F16, tag=f"po{bi}",
                                        name="po")
                        nc.vector.tensor_scalar(po, pb, rec, OUT_SCALE,
                                                op0=OP.mult, op1=OP.mult)
                    elif tail_psum:
                        # DMA cannot source PSUM; land the result in SBUF
                        po = epool.tile([128, ch], F32, tag=f"po{bi}",
                                        name="po")
                        nc.vector.tensor_scalar_mul(po, pb, rec)
                    else:
                        po = pb
                        nc.vector.tensor_scalar_mul(pb, pb, rec)
                    nc.scalar.dma_start(
                        out=attn[b].rearrange("(p c) -> p c", c=ch),
                        in_=po)

        if hw_loop > 1:
            with tc.For_i(0, hw_loop):
                body()
        else:
            body()
    nc.compile()
    return nc


def _host_prep(x, a, node_index, adj_mask):
    x = np.asarray(x, dtype=np.float32)
    a = np.asarray(a, dtype=np.float32).reshape(2 * D)
    adj = np.asarray(adj_mask)
    idx = int(node_index)
    a_tgt, a_src = a[:D], a[D:]

    tgt = (x[:, idx, :] @ a_tgt).astype(np.float32)          # [B]

    keep = np.flatnonzero(adj > 0)                           # [M] node ids
    m = len(keep)
    assert m > 0, "all-masked adjacency not supported"
    ch = max(CH, -(-m // 128))                               # capacity chunks
    padc = 128 * ch

    # col = c*128 + p  <->  compact slot k = p*ch + c
    kk = (np.arange(ch)[:, None] + np.arange(128)[None, :] * ch).ravel()
    valid = kk < m

    # xh[pair, bi*64+d, col] = x[2*pair+bi, keep[kk[col]], d]  (fp16)
    xt16 = np.ascontiguousarray(x.transpose(0, 2, 1), dtype=np.float16)
    xh = xt16[:, :, keep[np.minimum(kk, m - 1)]]             # [B, D, padc]
    w = (a_src * (NEGW / max(float(a_src @ a_src), 1e-12))).astype(np.float16)
    xh[:, :, ~valid] = w[None, :, None]
    xh = np.ascontiguousarray(xh).reshape(B // 2, 128, padc)

    amov = np.zeros((128, 2), np.float16)
    amov[0:D, 0] = a_src
    amov[D:2 * D, 1] = a_src

    return xh, amov, tgt, keep, ch


def _in_maps(xh, amov, tgt, keep, ch):
    maps = []
    for c in range(NCORES):
        tgtv = np.ascontiguousarray(np.broadcast_to(
            tgt[c * BPC:(c + 1) * BPC][None, :], (128, BPC)).astype(
                np.float32))
        maps.append({
            "xs": xh[c * PAIRS:(c + 1) * PAIRS],
            "amov": amov,
            "tgtv": tgtv,
        })
    return maps


def _runner(ch=CH):
    """Build the Bass program once and wrap its NEFF custom call in a jitted
    shard_map over the 8 cores."""
    key = ("runner", ch)
    if key in _CACHE:
        return _CACHE[key]
    install_neuronx_cc_hook()
    nc = _CACHE.setdefault(("nc", ch), _build(ch=ch))
    in_names, out_names, out_avals, zero_shapes = [], [], [], []
    for alloc in nc.m.functions[0].allocations:
        if not isinstance(alloc, mybir.MemoryLocationSet):
            continue
        name = alloc.memorylocations[0].name
        if alloc.kind == "ExternalInput":
            in_names.append(name)
        elif alloc.kind == "ExternalOutput":
            out_names.append(name)
            shape = tuple(alloc.tensor_shape)
            dtype = mybir.dt.np(alloc.dtype)
            out_avals.append(jax.core.ShapedArray(shape, dtype))
            zero_shapes.append((shape, dtype))

    def _body(*args):
        return tuple(_bass_exec_p.bind(
            *args,
            out_avals=tuple(out_avals),
            in_names=tuple(in_names + out_names),
            out_names=tuple(out_names),
            lowering_input_output_aliases=(),
            sim_require_finite=True,
            sim_require_nnan=True,
            nc=nc,
        ))

    mesh = Mesh(np.asarray(jax.devices()[:NCORES]), ("core",))
    nin = len(in_names) + len(out_names)
    sharded = jax.jit(shard_map(
        _body, mesh=mesh,
        in_specs=(PartitionSpec("core"),) * nin,
        out_specs=(PartitionSpec("core"),) * len(out_names),
        check_rep=False))
    _CACHE[key] = (sharded, in_names, out_names, zero_shapes)
    return _CACHE[key]


def kernel(x, a, node_index, adj_mask):
    global LAST_RUN
    prep = _host_prep(x, a, node_index, adj_mask)
    ch, keep, m = prep[4], prep[3], len(prep[3])
    maps = _in_maps(*prep)
    sharded, in_names, out_names, zero_shapes = _runner(ch=ch)
    # concat of the 8 per-core xs shards is exactly the full array
    ins = [prep[0] if nm == "xs" else
           np.concatenate([mp[nm] for mp in maps], axis=0)
           for nm in in_names]
    zeros = [np.zeros((NCORES * s[0], *s[1:]), d) for s, d in zero_shapes]
    outs = sharded(*ins, *zeros)
    LAST_RUN = outs
    attn_c = np.asarray(outs[out_names.index("attn")])  # [B, 128*ch]
    full = np.zeros((B, N), np.float32)
    if attn_c.dtype == np.float16:
        full[:, keep] = attn_c[:, :m].astype(np.float32) * (1.0 / OUT_SCALE)
    else:
        full[:, keep] = attn_c[:, :m]
    return full
